# revision 3
# baseline (speedup 1.0000x reference)
"""Trainium2 Bass kernel for KVCache.update.

Semantics (matching the reference):
  - scatter xk/xv into k_cache/v_cache at [layer_idx, :, curr_pos:curr_pos+S]
    producing fresh full-cache outputs k_new/v_new
  - gather the prefix [:curr_pos+S] of the updated layer and
    repeat_interleave KV heads by n_rep (GQA) producing keys/values

Sharding: tensor-parallel over the 8 KV heads (1 head per NeuronCore).
Each core:
  - copies its (LAYERS, BSZ, MAX_SEQ, HD) cache shard HBM->HBM with two
    seq_len*HD holes left at the scatter target, then DMAs xk/xv into the
    holes (all writes disjoint -> no ordering hazards)
  - gathers the layer_idx prefix once into SBUF (partition = 16-row chunk
    so the store is fully contiguous in DRAM), replicates x n_rep with
    on-chip copies, and stores keys/values without re-reading HBM n_rep
    times.
"""

import numpy as np
import ml_dtypes

N_CORES = 8

_BUILD_CACHE = {}


def _build_bass(layers, bsz, max_seq, hd, seq_len, layer_idx, curr_pos, n_rep):
    import concourse.bass as bass
    import concourse.mybir as mybir

    dt = mybir.dt.bfloat16
    total = curr_pos + seq_len
    nc = bass.Bass()

    kc = nc.declare_dram_parameter("kc", [layers, bsz, max_seq, hd], dt, isOutput=False)
    vc = nc.declare_dram_parameter("vc", [layers, bsz, max_seq, hd], dt, isOutput=False)
    xk = nc.declare_dram_parameter("xk", [bsz, seq_len, hd], dt, isOutput=False)
    xv = nc.declare_dram_parameter("xv", [bsz, seq_len, hd], dt, isOutput=False)
    k_new = nc.declare_dram_parameter(
        "k_new", [layers, bsz, max_seq, hd], dt, isOutput=True
    )
    v_new = nc.declare_dram_parameter(
        "v_new", [layers, bsz, max_seq, hd], dt, isOutput=True
    )
    keys = nc.declare_dram_parameter("keys", [bsz, total, n_rep, hd], dt, isOutput=True)
    values = nc.declare_dram_parameter(
        "values", [bsz, total, n_rep, hd], dt, isOutput=True
    )

    def emit(eng, cache, x, new, out, sem):
        """All-DMA program for one cache; every DMA is independent."""
        n = 0
        # gather: keys[b, t, r, :] <- prefix/x row, with the n_rep repeat
        # expressed as a step-0 broadcast dim on the DMA *source* so the
        # destination (which is contiguous) is written at line rate.
        for b in range(bsz):
            src = (
                cache[layer_idx, b, 0:curr_pos, :]
                .unsqueeze(1)
                .broadcast_to((curr_pos, n_rep, hd))
            )
            eng.dma_start(out[b, 0:curr_pos, :, :], src).then_inc(sem, 16)
            n += 1
            tail = x[b].unsqueeze(1).broadcast_to((seq_len, n_rep, hd))
            eng.dma_start(out[b, curr_pos:total, :, :], tail).then_inc(sem, 16)
            n += 1
        # full-shard HBM->HBM copy, skipping the scatter holes
        flat_src = cache[:].rearrange("a b c d -> (a b c d)")
        flat_dst = new[:].rearrange("a b c d -> (a b c d)")
        n_elems = layers * bsz * max_seq * hd
        hole_len = seq_len * hd
        pos = 0
        for b in range(bsz):
            h = ((layer_idx * bsz + b) * max_seq + curr_pos) * hd
            if h > pos:
                eng.dma_start(flat_dst[pos:h], flat_src[pos:h]).then_inc(sem, 16)
                n += 1
            pos = h + hole_len
        if n_elems > pos:
            eng.dma_start(flat_dst[pos:n_elems], flat_src[pos:n_elems]).then_inc(
                sem, 16
            )
            n += 1
        # scatter the new tokens into the holes
        eng.dma_start(new[layer_idx, :, curr_pos:total, :], x[:]).then_inc(sem, 16)
        n += 1
        eng.wait_ge(sem, 16 * n)

    with (
        nc.Block() as block,
        nc.semaphore("sem_k") as sem_k,
        nc.semaphore("sem_v") as sem_v,
    ):

        @block.sync
        def _(sync):
            emit(sync, kc, xk, k_new, keys, sem_k)

        @block.scalar
        def _(scalar):
            emit(scalar, vc, xv, v_new, values, sem_v)

    return nc


def _get_bass(key):
    if key not in _BUILD_CACHE:
        _BUILD_CACHE[key] = _build_bass(*key)
    return _BUILD_CACHE[key]


def _as_bf16(a):
    a = np.asarray(a)
    if a.dtype != ml_dtypes.bfloat16:
        a = a.astype(ml_dtypes.bfloat16)
    return a


def _run(k_cache, v_cache, xk, xv, layer_idx, curr_pos, n_rep, **spmd_kwargs):
    layer_idx = int(layer_idx)
    curr_pos = int(curr_pos)
    n_rep = int(n_rep)

    k_cache = _as_bf16(k_cache)
    v_cache = _as_bf16(v_cache)
    xk = _as_bf16(xk)
    xv = _as_bf16(xv)

    layers, bsz, max_seq, n_kv, hd = k_cache.shape
    seq_len = xk.shape[1]
    assert n_kv == N_CORES, f"expected {N_CORES} KV heads, got {n_kv}"

    nc = _get_bass((layers, bsz, max_seq, hd, seq_len, layer_idx, curr_pos, n_rep))

    in_maps = []
    for c in range(N_CORES):
        in_maps.append(
            {
                "kc": np.ascontiguousarray(k_cache[:, :, :, c, :]),
                "vc": np.ascontiguousarray(v_cache[:, :, :, c, :]),
                "xk": np.ascontiguousarray(xk[:, :, c, :]),
                "xv": np.ascontiguousarray(xv[:, :, c, :]),
            }
        )

    from concourse.bass_utils import run_bass_kernel_spmd

    res = run_bass_kernel_spmd(nc, in_maps, core_ids=list(range(N_CORES)), **spmd_kwargs)
    rs = res.results

    k_new = np.stack([r["k_new"] for r in rs], axis=3)
    v_new = np.stack([r["v_new"] for r in rs], axis=3)
    keys = np.concatenate([r["keys"] for r in rs], axis=2)
    values = np.concatenate([r["values"] for r in rs], axis=2)
    return (keys, values, k_new, v_new), res


def kernel(k_cache, v_cache, xk, xv, layer_idx, curr_pos, n_rep):
    outs, _ = _run(k_cache, v_cache, xk, xv, layer_idx, curr_pos, n_rep)
    return outs
